# revision 4
# baseline (speedup 1.0000x reference)
"""Tensor-parallel multi-head attention for Trainium2 (8 NeuronCores).

Problem: x:[2,16,2048,1024], wq/wk/wv:[64,1024], wo:[1024,1024]
  xq/xk/xv = einsum('bhsd,kd->bhsk', x, w)          (per-head, shared w)
  score    = xq @ xk.T / sqrt(1024); attn = softmax(score)
  out      = (attn @ xv) -> [B,S,H*dk] @ wo.T -> [B,S,1024]

Sharding: head-parallel over 8 cores (2 heads/core x 2 batches = 4
(b,h) pairs per core). Each core computes its heads' attention in
transposed layout (out.T blocks, rows = head dk), AllGathers the small
activation blocks, then computes a 128-column slice of the output
projection (row-shard of wo.T) -- no all-reduce needed.

Performance structure (v2):
- The exp stream on ScalarE (128 ACTIVATEs of [128,1024], ~1.11us each)
  is the roofline; the t-loop is software-pipelined so scores(t+1) is
  emitted BEFORE attnv(t), keeping ACT back-to-back instead of the
  serial score->exp->attnv chain.
- All other PE work (v projection, vt transposes, next pair's q/k
  projection, batch-0 output projection) is emitted as per-t fillers
  sized to fit the ACT-period PE budget.
- v projection col-tiled: two n-chunks run concurrently in the two
  64-column halves of the PE array (wv duplicated into both halves).
- q/k duplication done with 3 PSUM casts instead of 4.
- softmax denominator via an all-ones column appended to V (row 64 of
  the attn@V accumulator); normalization on the [64, S] output only.
- AllGather per (b,h) pair overlapped with the next pair's attention;
  last pair gathered in half-column chunks; batch-1 output projection
  split by contraction rows so only the last gather's rows trail.
"""

import os
import sys

import numpy as np

sys.path.insert(0, "/opt/trn_rl_repo")

# Prefer Mesh over RDH for the small AllGathers (RDH measured ~33us for
# 256KB/rank here; each op serializes on the single cc stream).
os.environ.setdefault("NEURON_RT_DBG_RDH_CC", "0")

import ml_dtypes  # noqa: E402

import concourse.bass as bass  # noqa: E402
import concourse.mybir as mybir  # noqa: E402
import concourse.tile as tile  # noqa: E402
from concourse import bacc  # noqa: E402
from concourse.bass_utils import run_bass_kernel_spmd  # noqa: E402
from concourse.masks import make_identity  # noqa: E402

N_CORES = 8
B, H, S, D = 2, 16, 2048, 1024
DK = D // H            # 64
HPC = H // N_CORES     # heads per core = 2
PAIRS = B * HPC        # (b, h) pairs per core = 4
SC = 512               # s-chunk (PSUM free-dim limit for f32)
NSC = S // SC          # 4 s-chunks per pair
NT = S // 128          # 16 t-tiles
NDC = D // 128         # 8 contraction chunks of 128
BS = B * S             # 4096 flattened (b, s) columns
INV_SCALE = 1.0 / 32.0  # 1/sqrt(D)

F32 = mybir.dt.float32
BF16 = mybir.dt.bfloat16

_GRAPH = None
LAST_RESULTS = None  # BassKernelResults of the most recent run (for test.py)


def _build_graph():
    nc = bacc.Bacc("TRN2", target_bir_lowering=False, num_devices=N_CORES)

    xt = nc.declare_dram_parameter("xt", [PAIRS, D, S], BF16, isOutput=False)
    wqk = nc.declare_dram_parameter("wqk", [D, 128], BF16, isOutput=False)
    wv2 = nc.declare_dram_parameter("wv2", [D, 128], BF16, isOutput=False)
    wo = nc.declare_dram_parameter("wo", [D, 128], BF16, isOutput=False)
    out = nc.declare_dram_parameter("out", [128, BS], F32, isOutput=True)

    Exp = mybir.ActivationFunctionType.Exp

    with tile.TileContext(nc) as tc:
        with (
            tc.tile_pool(name="const", bufs=1) as cpool,
            tc.tile_pool(name="dram", bufs=1, space="DRAM") as dpool,
            tc.tile_pool(name="xin", bufs=2) as xpool,
            tc.tile_pool(name="qkv", bufs=2) as qkvpool,
            tc.tile_pool(name="vtiles", bufs=2) as vpool,
            tc.tile_pool(name="exp", bufs=3) as epool,
            tc.tile_pool(name="norm", bufs=2) as npool,
            tc.tile_pool(name="aio", bufs=1) as apool,
            tc.tile_pool(name="oout", bufs=2) as opool,
            tc.tile_pool(name="ps_proj", bufs=2, space="PSUM") as ps_proj,
            tc.tile_pool(name="ps_sc", bufs=2, space="PSUM") as ps_sc,
            tc.tile_pool(name="ps_ou", bufs=1, space="PSUM") as ps_ou,
        ):
            # Weights, bf16, laid out [128 partitions, chunk, m]
            wqk_sb = cpool.tile([128, NDC, 128], BF16)
            nc.sync.dma_start(
                out=wqk_sb[:], in_=wqk[:].rearrange("(c p) m -> p c m", p=128)
            )
            wv2_sb = cpool.tile([128, NDC, 128], BF16)
            nc.sync.dma_start(
                out=wv2_sb[:], in_=wv2[:].rearrange("(c p) m -> p c m", p=128)
            )
            wo_sb = cpool.tile([128, NDC, 128], BF16)
            nc.sync.dma_start(
                out=wo_sb[:], in_=wo[:].rearrange("(c p) m -> p c m", p=128)
            )
            ident64 = cpool.tile([64, 64], BF16)
            make_identity(nc, ident64[:])

            # Collective bounce buffers: one chunk per (b, h) pair
            ag_in4 = dpool.tile([PAIRS, DK, S], BF16)
            ag_out4 = [
                dpool.tile(
                    [N_CORES, DK, S], BF16, addr_space="Shared",
                    name=f"ag_out_p{p}",
                )
                for p in range(PAIRS - 1)
            ]
            warm_in = dpool.tile([64, 16], BF16)
            warm_out = dpool.tile(
                [N_CORES, 64, 16], BF16, addr_space="Shared", name="warm_out"
            )
            # last pair gathered in two half-column chunks (shorter tail)
            ag_in_h = dpool.tile([2, DK, S // 2], BF16)
            ag_out_h = [
                dpool.tile(
                    [N_CORES, DK, S // 2], BF16, addr_space="Shared",
                    name=f"ag_out_h{g}",
                )
                for g in range(2)
            ]

            nc.vector.memset(
                warm_in_sb0 := cpool.tile([64, 16], BF16, name="warm_sb"), 0.0
            )
            nc.sync.dma_start(out=warm_in[:], in_=warm_in_sb0[:])
            nc.gpsimd.collective_compute(
                "AllGather",
                mybir.AluOpType.bypass,
                replica_groups=[list(range(N_CORES))],
                ins=[warm_in.opt()],
                outs=[warm_out.opt()],
            )

            asb_tiles = {}

            def get_asb(b):
                if b not in asb_tiles:
                    asb_tiles[b] = apool.tile(
                        [128, NDC, S], BF16, tag="asb", name=f"asb{b}"
                    )
                return asb_tiles[b]

            def outproj_dma(b, ns, rows=("lower", "upper")):
                """Load the gathered activation rows for batch b's columns."""
                asb = get_asb(b)
                lo, hi = min(ns) * SC, (max(ns) + 1) * SC
                for c in range(NDC):
                    if "lower" in rows:
                        nc.sync.dma_start(
                            out=asb[0:64, c, lo:hi],
                            in_=ag_out4[HPC * b][c][:, lo:hi],
                        )
                    if "upper" not in rows:
                        continue
                    if HPC * b + 1 < PAIRS - 1:
                        nc.sync.dma_start(
                            out=asb[64:128, c, lo:hi],
                            in_=ag_out4[HPC * b + 1][c][:, lo:hi],
                        )
                    else:
                        g = lo // (S // 2)
                        h0 = g * (S // 2)
                        nc.sync.dma_start(
                            out=asb[64:128, c, lo:hi],
                            in_=ag_out_h[g][c][:, lo - h0 : hi - h0],
                        )

            def outproj(b, ns, split=False, pool_tag=None, do_dma=True):
                """Output projection for batch b's columns, s-chunks `ns`.
                split=True emits the lower 64 contraction rows first so they
                run before the upper rows' AllGather completes."""
                if do_dma:
                    outproj_dma(b, ns)
                asb = get_asb(b)
                oscope = nc.named_scope(f"outproj{b}_{min(ns)}")
                oscope.__enter__()
                pool, tag = (ps_sc, "sc") if pool_tag == "sc" else (
                    ps_proj, "proj_ps"
                )
                o_tiles = {}
                strips = ((0, 64), (64, 128)) if split else ((0, 128),)
                for si, (r0, r1) in enumerate(strips):
                    for n in ns:
                        if n not in o_tiles:
                            o_tiles[n] = pool.tile(
                                [128, SC], F32, tag=tag, name=f"o_ps{b}_{n}"
                            )
                        o_ps = o_tiles[n]
                        for c in range(NDC):
                            nc.tensor.matmul(
                                o_ps[:],
                                wo_sb[r0:r1, c, :],
                                asb[r0:r1, c, n * SC : (n + 1) * SC],
                                start=(c == 0 and si == 0),
                                stop=(
                                    c == NDC - 1 and si == len(strips) - 1
                                ),
                                tile_position=(r0, 0),
                            )
                for n in ns:
                    o_sb = opool.tile([128, SC], F32, tag="o_sb")
                    nc.vector.tensor_copy(o_sb[:], o_tiles[n][:])
                    nc.sync.dma_start(
                        out=out[:, b * S + n * SC : b * S + (n + 1) * SC],
                        in_=o_sb[:],
                    )
                oscope.__exit__(None, None, None)

            def emit_xT(p):
                # n-major sub-block loads: early chunks land after ~1MB
                xT = xpool.tile([128, NDC, S], BF16, tag="xT", name=f"xT{p}")
                for n in range(NSC):
                    for c in range(NDC):
                        nc.sync.dma_start(
                            out=xT[:, c, n * SC : (n + 1) * SC],
                            in_=xt[p][
                                c * 128 : (c + 1) * 128, n * SC : (n + 1) * SC
                            ],
                        )
                return xT

            def alloc_qk(p):
                # qk: partitions 0:64 = q, 64:128 = k
                # qk2: partitions 0:64 = k, 64:128 = q  (for strip alternation)
                qk_sb = qkvpool.tile([128, S], BF16, tag="qk", name=f"qk{p}")
                qk2_sb = qkvpool.tile([128, S], BF16, tag="qk2", name=f"qk2{p}")
                return qk_sb, qk2_sb

            def emit_qk(xT, qk_sb, qk2_sb, n):
                """One n-chunk of the merged q/k projection + duplication."""
                nsl = slice(n * SC, (n + 1) * SC)
                ps_qk = ps_proj.tile([128, SC], F32, tag="proj_ps", name="ps_qk")
                for c in range(NDC):
                    nc.tensor.matmul(
                        ps_qk[:],
                        wqk_sb[:, c, :],
                        xT[:, c, nsl],
                        start=(c == 0),
                        stop=(c == NDC - 1),
                    )
                nc.vector.tensor_copy(qk_sb[:, nsl], ps_qk[:])
                nc.vector.tensor_copy(qk2_sb[0:64, nsl], ps_qk[64:128, :])
                nc.vector.tensor_copy(qk2_sb[64:128, nsl], ps_qk[0:64, :])

            def alloc_v(p):
                vT_sb = qkvpool.tile([64, S], BF16, tag="vT")
                vt = [
                    vpool.tile([128, 65], BF16, tag=f"vt{t}", name=f"vt{t}")
                    for t in range(NT)
                ]
                return vT_sb, vt

            def v_mm(xT, vT_sb, jj):
                """v projection for chunk pair (2jj, 2jj+1), col-tiled so the
                two chunks stream concurrently in the two array halves."""
                sla = slice(2 * jj * SC, (2 * jj + 1) * SC)
                slb = slice((2 * jj + 1) * SC, (2 * jj + 2) * SC)
                ps_v = ps_proj.tile([128, SC], F32, tag="proj_ps", name="ps_v")
                for c in range(NDC):
                    nc.tensor.matmul(
                        ps_v[0:64, :],
                        wv2_sb[:, c, 0:64],
                        xT[:, c, sla],
                        start=(c == 0),
                        stop=(c == NDC - 1),
                        tile_position=(0, 0),
                    )
                    nc.tensor.matmul(
                        ps_v[64:128, :],
                        wv2_sb[:, c, 64:128],
                        xT[:, c, slb],
                        start=(c == 0),
                        stop=(c == NDC - 1),
                        tile_position=(0, 64),
                    )
                nc.vector.tensor_copy(vT_sb[:, sla], ps_v[0:64, :])
                nc.vector.tensor_copy(vT_sb[:, slb], ps_v[64:128, :])

            def v_tp(vT_sb, vt, t0):
                """PE transposes for vt tiles t0..t0+3 (+ ones column)."""
                vt_q = ps_proj.tile(
                    [128, 4, 64], BF16, tag="proj_ps", name="vt_q"
                )
                for tt in range(t0, t0 + 4):
                    nc.tensor.transpose(
                        vt_q[:, tt - t0, :],
                        vT_sb[:, tt * 128 : (tt + 1) * 128],
                        ident64[:],
                    )
                    nc.vector.memset(vt[tt][:, 64:65], 1.0)
                    nc.vector.tensor_copy(
                        vt[tt][:, 0:64], vt_q[:, tt - t0, :]
                    )

            def emit_scores(qk_sb, qk2_sb, half, t):
                """Score matmuls for t-tile t, both n-chunks of this half,
                on alternating 64-row strips (concurrent execution)."""
                tsl = slice(t * 128, (t + 1) * 128)
                sc = ps_sc.tile([128, 2 * SC], F32, tag="sc", name="sc")
                for i in range(2):
                    n = 2 * half + i
                    nsl = slice(n * SC, (n + 1) * SC)
                    if n % 2 == 0:
                        nc.tensor.matmul(
                            sc[:, i * SC : (i + 1) * SC],
                            qk2_sb[0:64, tsl],
                            qk_sb[0:64, nsl],
                            start=True,
                            stop=True,
                            tile_position=(0, 0),
                        )
                    else:
                        nc.tensor.matmul(
                            sc[:, i * SC : (i + 1) * SC],
                            qk_sb[64:128, tsl],
                            qk2_sb[64:128, nsl],
                            start=True,
                            stop=True,
                            tile_position=(64, 0),
                        )
                return sc

            def run_half(p, half, qk_sb, qk2_sb, vt, fillers, sc0):
                """Software-pipelined t-loop: ACT(t) and scores(t+1) are
                emitted before attnv(t) so the exp stream never waits on
                the attn@v matmuls."""
                ascope = nc.named_scope(f"attn{p}h{half}")
                ascope.__enter__()
                ou = [
                    ps_ou.tile([65, SC], F32, tag=f"ou{i}", name=f"ou{i}")
                    for i in range(2)
                ]
                sc_cur = sc0
                for t in range(NT):
                    pexp = epool.tile([128, 2 * SC], BF16, tag="pexp")
                    nc.scalar.activation(
                        pexp[:], sc_cur[:], Exp, scale=INV_SCALE
                    )
                    if t + 1 < NT:
                        sc_cur = emit_scores(qk_sb, qk2_sb, half, t + 1)
                    for f in fillers.get(t, ()):
                        f()
                    for i in range(2):
                        nc.tensor.matmul(
                            ou[i][:],
                            vt[t][:],
                            pexp[:, i * SC : (i + 1) * SC],
                            start=(t == 0),
                            stop=(t == NT - 1),
                        )
                # normalize and ship the two finished n-chunks
                for i in range(2):
                    n = 2 * half + i
                    ou_sb = npool.tile([65, SC], F32, tag="ou_sb")
                    nc.vector.tensor_copy(ou_sb[:], ou[i][:])
                    den0 = npool.tile([1, SC], F32, tag="den0")
                    nc.vector.tensor_copy(den0[0:1, :], ou_sb[64:65, :])
                    recip = npool.tile([1, SC], F32, tag="recip")
                    nc.vector.reciprocal_approx_fast(
                        recip[0:1, :], den0[0:1, :]
                    )
                    bcast = npool.tile([64, SC], F32, tag="bcast")
                    nc.gpsimd.partition_broadcast(bcast[:], recip[0:1, :])
                    onorm = npool.tile([64, SC], BF16, tag="onorm")
                    nc.vector.tensor_mul(
                        onorm[:], ou_sb[0:64, :], bcast[:]
                    )
                    if p < PAIRS - 1:
                        nc.sync.dma_start(
                            out=ag_in4[p][:, n * SC : (n + 1) * SC],
                            in_=onorm[:],
                        )
                    else:
                        nc.sync.dma_start(
                            out=ag_in_h[n // 2][
                                :, (n % 2) * SC : (n % 2 + 1) * SC
                            ],
                            in_=onorm[:],
                        )
                ascope.__exit__(None, None, None)
                if p == PAIRS - 1:
                    nc.gpsimd.collective_compute(
                        "AllGather",
                        mybir.AluOpType.bypass,
                        replica_groups=[list(range(N_CORES))],
                        ins=[ag_in_h[half].opt()],
                        outs=[ag_out_h[half].opt()],
                    )

            # ~3.5us of dummy matmuls during the initial DMA window so the
            # PE clock gate (HAM) is already open when real work issues.
            warm_ps = ps_proj.tile([64, SC], F32, tag="proj_ps", name="warm_ps")
            for w in range(8):
                nc.tensor.matmul(
                    warm_ps[:],
                    wqk_sb[:, 0, 0:64],
                    wqk_sb[:, 4 * (w % 2) : 4 * (w % 2) + 4, :],
                    start=True,
                    stop=True,
                )

            # ---- pair-0 prologue ----
            xT_t = {0: emit_xT(0)}
            qk_t = {0: alloc_qk(0)}
            v_t = {0: alloc_v(0)}
            emit_qk(xT_t[0], *qk_t[0], 0)
            emit_qk(xT_t[0], *qk_t[0], 1)

            for p in range(PAIRS):
                xT = xT_t.pop(p)
                qk_sb, qk2_sb = qk_t.pop(p)
                vT_sb, vt = v_t.pop(p)

                # boundary: first scores, then this pair's first v chunk-pair
                sc0 = emit_scores(qk_sb, qk2_sb, 0, 0)
                v_mm(xT, vT_sb, 0)

                f_h0 = {
                    0: [lambda: v_tp(vT_sb, vt, 0)],
                    1: [lambda: v_tp(vT_sb, vt, 4)],
                    2: [lambda: v_mm(xT, vT_sb, 1)],
                    4: [lambda: v_tp(vT_sb, vt, 8)],
                    5: [lambda: v_tp(vT_sb, vt, 12)],
                }
                if p == 0:
                    f_h0[3] = [lambda: emit_qk(xT, qk_sb, qk2_sb, 2)]
                    f_h0[6] = [lambda: emit_qk(xT, qk_sb, qk2_sb, 3)]
                if p + 1 < PAIRS:
                    f_h0[8] = [lambda: xT_t.__setitem__(p + 1, emit_xT(p + 1))]
                if p == 2:
                    # batch-0 lower activation rows (pair 0, AG long done)
                    f_h0[10] = [lambda: outproj_dma(0, [0, 1, 2, 3],
                                                    rows=("lower",))]
                run_half(p, 0, qk_sb, qk2_sb, vt, f_h0, sc0)

                sc0 = emit_scores(qk_sb, qk2_sb, 1, 0)
                f_h1 = {}
                if p + 1 < PAIRS:
                    qk_t[p + 1] = alloc_qk(p + 1)
                    v_t[p + 1] = alloc_v(p + 1)
                    xTn = xT_t[p + 1]
                    qkn, qk2n = qk_t[p + 1]
                    for n in range(NSC):
                        f_h1[2 * n] = [
                            lambda n=n: emit_qk(xTn, qkn, qk2n, n)
                        ]
                if p == 2:
                    # batch-0 upper activation rows (pair 1, AG done by now)
                    f_h1[9] = [lambda: outproj_dma(0, [0, 1, 2, 3],
                                                   rows=("upper",))]
                if p == PAIRS - 1:
                    # batch-0 output projection fills the PE idle slots
                    f_h1[0] = [lambda: outproj(0, [0, 1], do_dma=False)]
                    f_h1[4] = [lambda: outproj(0, [2, 3], do_dma=False)]
                run_half(p, 1, qk_sb, qk2_sb, vt, f_h1, sc0)

                # AllGather this pair's activation block (overlaps the next
                # pair's compute). Last pair gathered per-half in run_half.
                if p < PAIRS - 1:
                    nc.gpsimd.collective_compute(
                        "AllGather",
                        mybir.AluOpType.bypass,
                        replica_groups=[list(range(N_CORES))],
                        ins=[ag_in4[p].opt()],
                        outs=[ag_out4[p].opt()],
                    )

            # ---- batch-1 output projection tail ----
            outproj_dma(1, [0, 1, 2, 3], rows=("lower",))
            outproj_dma(1, [0, 1], rows=("upper",))
            outproj(1, [0, 1], split=True, do_dma=False)
            outproj_dma(1, [2, 3], rows=("upper",))
            outproj(1, [2, 3], split=True, pool_tag="sc", do_dma=False)

    return nc


def _get_graph():
    global _GRAPH
    if _GRAPH is None:
        _GRAPH = _build_graph()
        if not _GRAPH.is_finalized():
            _GRAPH.finalize()
    return _GRAPH


def kernel(x, wq, wk, wv, wo):
    global LAST_RESULTS
    x = np.asarray(x, dtype=np.float32)
    wq = np.asarray(wq, dtype=np.float32)
    wk = np.asarray(wk, dtype=np.float32)
    wv = np.asarray(wv, dtype=np.float32)
    wo = np.asarray(wo, dtype=np.float32)

    bf16 = ml_dtypes.bfloat16
    # x transposed to [B, H, D, S] once (feeds matmuls as the moving operand)
    xt_all = np.ascontiguousarray(x.transpose(0, 1, 3, 2)).astype(bf16)
    wqk_t = np.ascontiguousarray(
        np.concatenate([wq, wk], axis=0).T
    ).astype(bf16)  # [D, 128]
    wv_t = np.ascontiguousarray(wv.T).astype(bf16)  # [D, 64]
    wv2_t = np.ascontiguousarray(
        np.concatenate([wv_t, wv_t], axis=1)
    )  # [D, 128] duplicated for col-tiled v projection
    wo_t = np.ascontiguousarray(wo.T).astype(bf16)  # [D, D]; cols sliced per core

    in_maps = []
    for r in range(N_CORES):
        h0 = HPC * r
        # pair order: p = b*HPC + hl -> (b, h0+hl)
        xt_np = np.ascontiguousarray(
            xt_all[:, h0 : h0 + HPC].reshape(PAIRS, D, S)
        )
        in_maps.append(
            {
                "xt": xt_np,
                "wqk": wqk_t,
                "wv2": wv2_t,
                "wo": np.ascontiguousarray(wo_t[:, 128 * r : 128 * (r + 1)]),
            }
        )

    nc = _get_graph()
    trace = bool(os.environ.get("BASS_TRACE"))
    if trace:
        try:  # tracing needs the axon NTFF hook; fall back cleanly
            from antenv.axon_hooks import get_axon_ntff_profile_hook  # noqa: F401
        except ImportError:
            trace = False
    tk = {}
    tc_env = os.environ.get("TRACE_CORES")
    if tc_env:
        tk["trace_cores"] = [int(c) for c in tc_env.split(",")]
    LAST_RESULTS = run_bass_kernel_spmd(
        nc, in_maps, core_ids=list(range(N_CORES)), trace=trace, **tk
    )
    outs = [LAST_RESULTS.results[r]["out"] for r in range(N_CORES)]
    full_t = np.concatenate(outs, axis=0)  # [D, B*S]
    return np.ascontiguousarray(full_t.T).reshape(B, S, D)


# revision 21
# speedup vs baseline: 1.0770x; 1.0770x over previous
"""Tensor-parallel multi-head attention for Trainium2 (8 NeuronCores).

v3: AllToAll-based output projection.

Problem: x:[2,16,2048,1024], wq/wk/wv:[64,1024], wo:[1024,1024]
  xq/xk/xv = einsum('bhsd,kd->bhsk', x, w)          (per-head, shared w)
  score    = xq @ xk.T / sqrt(1024); attn = softmax(score)
  out      = (attn @ xv) -> [B,S,H*dk] @ wo.T -> [B,S,1024]

Sharding: attention is head-parallel (2 heads x 2 batches = 4 pairs per
core, processed batch-interleaved: (h0,b0),(h0,b1),(h1,b0),(h1,b1)).
The output projection is TOKEN-parallel: core j computes all 1024
output dims for (b,s)-column slice [512j, 512j+512). The activation
redistribution is two AllToAll ops (one per head slot, 512KB each,
mesh algorithm) instead of per-pair AllGathers (which ran RDH at
~35us each and serialized on the single cc stream).

Attention pipeline per (pair, half): the ScalarE exp stream
(ACTIVATE [128,1024], ~1.11us) is kept back-to-back by emitting
scores(t+1) before attnv(t); v projection (col-tiled pairs),
vt transposes, and the next pair's q/k projection fill PE idle slots.
Softmax denominator via an all-ones column appended to V.
"""

import os
import sys

import numpy as np

sys.path.insert(0, "/opt/trn_rl_repo")

import ml_dtypes  # noqa: E402

import concourse.bass as bass  # noqa: E402
import concourse.mybir as mybir  # noqa: E402
import concourse.tile as tile  # noqa: E402
from concourse import bacc  # noqa: E402
from concourse.bass_utils import run_bass_kernel_spmd  # noqa: E402
from concourse.masks import make_identity  # noqa: E402

N_CORES = 8
B, H, S, D = 2, 16, 2048, 1024
DK = D // H            # 64
HPC = H // N_CORES     # heads per core = 2
PAIRS = B * HPC        # (b, h) pairs per core = 4
SC = 512               # s-chunk (PSUM free-dim limit for f32)
NSC = S // SC          # 4 s-chunks per pair
NT = S // 128          # 16 t-tiles
NDC = D // 128         # 8 contraction chunks of 128
BS = B * S             # 4096 flattened (b, s) columns
INV_SCALE = 1.0 / 32.0  # 1/sqrt(D)

F32 = mybir.dt.float32
BF16 = mybir.dt.bfloat16

_GRAPH = None
LAST_RESULTS = None  # BassKernelResults of the most recent run (for test.py)


def _build_graph():
    nc = bacc.Bacc("TRN2", target_bir_lowering=False, num_devices=N_CORES)

    # pairs in processing order q: (head-slot hl, batch b) = divmod(q, 2)
    xt = nc.declare_dram_parameter("xt", [PAIRS, D, S], BF16, isOutput=False)
    wqk = nc.declare_dram_parameter("wqk", [D, 128], BF16, isOutput=False)
    wv2 = nc.declare_dram_parameter("wv2", [D, 128], BF16, isOutput=False)
    wo = nc.declare_dram_parameter("wo", [D, D], BF16, isOutput=False)
    out = nc.declare_dram_parameter("out", [NDC, 128, SC], F32, isOutput=True)

    Exp = mybir.ActivationFunctionType.Exp

    with tile.TileContext(nc) as tc:
        with (
            tc.tile_pool(name="const", bufs=1) as cpool,
            tc.tile_pool(name="dram", bufs=1, space="DRAM") as dpool,
            tc.tile_pool(name="xin", bufs=2) as xpool,
            tc.tile_pool(name="qkv", bufs=2) as qkvpool,
            tc.tile_pool(name="vtiles", bufs=2) as vpool,
            tc.tile_pool(name="exp", bufs=3) as epool,
            tc.tile_pool(name="norm", bufs=2) as npool,
            tc.tile_pool(name="aio", bufs=1) as apool,
            tc.tile_pool(name="oout", bufs=2) as opool,
            tc.tile_pool(name="ps_proj", bufs=2, space="PSUM") as ps_proj,
            tc.tile_pool(name="ps_sc", bufs=2, space="PSUM") as ps_sc,
            tc.tile_pool(name="ps_ou", bufs=1, space="PSUM") as ps_ou,
        ):
            # Weights, bf16, laid out [128 partitions, chunk, m]
            wqk_sb = cpool.tile([128, NDC, 128], BF16)
            nc.sync.dma_start(
                out=wqk_sb[:], in_=wqk[:].rearrange("(c p) m -> p c m", p=128)
            )
            wv2_sb = cpool.tile([128, NDC, 128], BF16)
            nc.sync.dma_start(
                out=wv2_sb[:], in_=wv2[:].rearrange("(c p) m -> p c m", p=128)
            )
            # full output-projection weight: [128, c-chunk, m-tile, 128]
            wo_sb = cpool.tile([128, NDC, NDC, 128], BF16)
            nc.sync.dma_start(
                out=wo_sb[:],
                in_=wo[:].rearrange("(c p) (m w) -> p c m w", p=128, w=128),
            )
            ident64 = cpool.tile([64, 64], BF16)
            make_identity(nc, ident64[:])

            # AllToAll bounce buffers: two ops, one per head slot.
            # in[j] = this core's activations for dest core j's token slice.
            a2a_in = [
                dpool.tile([N_CORES, DK, SC], BF16, name=f"a2a_in{g}")
                for g in range(2)
            ]
            a2a_out = [
                dpool.tile([N_CORES, DK, SC], BF16, name=f"a2a_out{g}")
                for g in range(2)
            ]
            warm_in = dpool.tile([N_CORES, 8, 16], BF16)
            warm_out = dpool.tile([N_CORES, 8, 16], BF16, name="warm_out")

            # Warmup collective: triggered as early as possible so the
            # one-time collective-runtime init overlaps the ramp.
            nc.vector.memset(
                warm_in_sb0 := cpool.tile([8, 8 * 16], BF16, name="warm_sb"),
                0.0,
            )
            nc.sync.dma_start(
                out=warm_in[:],
                in_=warm_in_sb0[:].rearrange("a (c m) -> a c m", c=8),
            )
            nc.gpsimd.collective_compute(
                "AllToAll",
                mybir.AluOpType.bypass,
                replica_groups=[list(range(N_CORES))],
                ins=[warm_in.opt()],
                outs=[warm_out.opt()],
            )

            # gathered activations for my token slice: [128, c, 512]
            # rows 0:64 of chunk c = head 2c (op 0), 64:128 = head 2c+1 (op 1)
            asb = apool.tile([128, NDC, SC], BF16, name="asb")

            def emit_xT(q):
                # n-major sub-block loads: early chunks land after ~1MB
                xT = xpool.tile([128, NDC, S], BF16, tag="xT", name=f"xT{q}")
                for n in range(NSC):
                    for c in range(NDC):
                        nc.sync.dma_start(
                            out=xT[:, c, n * SC : (n + 1) * SC],
                            in_=xt[q][
                                c * 128 : (c + 1) * 128, n * SC : (n + 1) * SC
                            ],
                        )
                return xT

            def alloc_qk(q):
                # qk: partitions 0:64 = q, 64:128 = k
                # qk2: partitions 0:64 = k, 64:128 = q (for strip alternation)
                qk_sb = qkvpool.tile([128, S], BF16, tag="qk", name=f"qk{q}")
                qk2_sb = qkvpool.tile([128, S], BF16, tag="qk2", name=f"qk2{q}")
                return qk_sb, qk2_sb

            qk_ps = {}

            def emit_qk(xT, qk_sb, qk2_sb, n):
                """One n-chunk of the merged q/k projection + duplication."""
                nsl = slice(n * SC, (n + 1) * SC)
                ps_qk = ps_proj.tile(
                    [128, SC], F32, tag="proj_ps", name="ps_qk"
                )
                for c in range(NDC):
                    nc.tensor.matmul(
                        ps_qk[:],
                        wqk_sb[:, c, :],
                        xT[:, c, nsl],
                        start=(c == 0),
                        stop=(c == NDC - 1),
                    )
                nc.vector.tensor_copy(qk_sb[:, nsl], ps_qk[:])
                nc.vector.tensor_copy(qk2_sb[0:64, nsl], ps_qk[64:128, :])
                nc.vector.tensor_copy(qk2_sb[64:128, nsl], ps_qk[0:64, :])

            def alloc_v(q):
                vT_sb = qkvpool.tile([64, S], BF16, tag="vT")
                vt = [
                    vpool.tile([128, 65], BF16, tag=f"vt{t}", name=f"vt{t}")
                    for t in range(NT)
                ]
                return vT_sb, vt

            def v_mm(xT, vT_sb, jj):
                """v projection for chunk pair (2jj, 2jj+1), col-tiled so
                the two chunks stream concurrently in the array halves."""
                sla = slice(2 * jj * SC, (2 * jj + 1) * SC)
                slb = slice((2 * jj + 1) * SC, (2 * jj + 2) * SC)
                ps_v = ps_proj.tile([128, SC], F32, tag="proj_ps", name="ps_v")
                for c in range(NDC):
                    nc.tensor.matmul(
                        ps_v[0:64, :],
                        wv2_sb[:, c, 0:64],
                        xT[:, c, sla],
                        start=(c == 0),
                        stop=(c == NDC - 1),
                        tile_position=(0, 0),
                        skip_group_check=True,
                    )
                    nc.tensor.matmul(
                        ps_v[64:128, :],
                        wv2_sb[:, c, 64:128],
                        xT[:, c, slb],
                        start=(c == 0),
                        stop=(c == NDC - 1),
                        tile_position=(0, 64),
                        skip_group_check=True,
                    )
                nc.vector.tensor_copy(vT_sb[:, sla], ps_v[0:64, :])
                nc.vector.tensor_copy(vT_sb[:, slb], ps_v[64:128, :])

            def v_tp(vT_sb, vt, t0, ntp=2):
                """PE transposes for vt tiles t0..t0+ntp (+ ones column)."""
                vt_q = ps_proj.tile(
                    [128, ntp, 64], BF16, tag="proj_ps", name="vt_q"
                )
                for tt in range(t0, t0 + ntp):
                    nc.tensor.transpose(
                        vt_q[:, tt - t0, :],
                        vT_sb[:, tt * 128 : (tt + 1) * 128],
                        ident64[:],
                    )
                    nc.vector.memset(vt[tt][:, 64:65], 1.0)
                    nc.vector.tensor_copy(
                        vt[tt][:, 0:64], vt_q[:, tt - t0, :]
                    )

            def emit_scores(qk_sb, qk2_sb, half, t):
                """Score matmuls for t-tile t, both n-chunks of this half,
                on alternating 64-row strips (concurrent execution)."""
                tsl = slice(t * 128, (t + 1) * 128)
                sc = ps_sc.tile([128, 2 * SC], F32, tag="sc", name="sc")
                for i in range(2):
                    n = 2 * half + i
                    nsl = slice(n * SC, (n + 1) * SC)
                    if n % 2 == 0:
                        nc.tensor.matmul(
                            sc[:, i * SC : (i + 1) * SC],
                            qk2_sb[0:64, tsl],
                            qk_sb[0:64, nsl],
                            start=True,
                            stop=True,
                            tile_position=(0, 0),
                        )
                    else:
                        nc.tensor.matmul(
                            sc[:, i * SC : (i + 1) * SC],
                            qk_sb[64:128, tsl],
                            qk2_sb[64:128, nsl],
                            start=True,
                            stop=True,
                            tile_position=(64, 0),
                        )
                return sc

            def run_half(q, half, qk_sb, qk2_sb, vt, fillers, sc0):
                """Software-pipelined t-loop: ACT(t) and scores(t+1) are
                emitted before attnv(t) so the exp stream never waits on
                the attn@v matmuls."""
                b = q % 2
                ascope = nc.named_scope(f"attn{q}h{half}")
                ascope.__enter__()
                ou = [
                    ps_ou.tile([65, SC], F32, tag=f"ou{i}", name=f"ou{i}")
                    for i in range(2)
                ]
                sc_cur = sc0
                last_mm = None
                for t in range(NT):
                    pexp = epool.tile([128, 2 * SC], BF16, tag="pexp")
                    nc.scalar.activation(
                        pexp[:], sc_cur[:], Exp, scale=INV_SCALE
                    )
                    if t + 1 < NT:
                        sc_cur = emit_scores(qk_sb, qk2_sb, half, t + 1)
                    for f in fillers.get(t, ()):
                        f()
                    for i in range(2):
                        last_mm = nc.tensor.matmul(
                            ou[i][:],
                            vt[t][:],
                            pexp[:, i * SC : (i + 1) * SC],
                            start=(t == 0),
                            stop=(t == NT - 1),
                        )
                # normalize and ship the two finished n-chunks to their
                # destination cores' slots of this head-slot's AllToAll
                for i in range(2):
                    n = 2 * half + i
                    ou_sb = npool.tile([65, SC], F32, tag="ou_sb")
                    nc.vector.tensor_copy(ou_sb[:], ou[i][:])
                    den0 = npool.tile([1, SC], F32, tag="den0")
                    nc.vector.tensor_copy(den0[0:1, :], ou_sb[64:65, :])
                    recip = npool.tile([1, SC], F32, tag="recip")
                    nc.vector.reciprocal_approx_fast(
                        recip[0:1, :], den0[0:1, :]
                    )
                    bcast = npool.tile([64, SC], F32, tag="bcast")
                    nc.gpsimd.partition_broadcast(bcast[:], recip[0:1, :])
                    onorm = npool.tile([64, SC], BF16, tag="onorm")
                    nc.vector.tensor_mul(
                        onorm[:], ou_sb[0:64, :], bcast[:]
                    )
                    nc.sync.dma_start(
                        out=a2a_in[q // 2][4 * b + n][:, :],
                        in_=onorm[:],
                    )
                ascope.__exit__(None, None, None)
                return last_mm

            # ~3.5us of dummy matmuls during the initial DMA window so the
            # PE clock gate (HAM) is already open when real work issues.
            warm_ps = ps_proj.tile([64, SC], F32, tag="proj_ps", name="warm_ps")
            for w in range(8):
                nc.tensor.matmul(
                    warm_ps[:],
                    wqk_sb[:, 0, 0:64],
                    wqk_sb[:, 4 * (w % 2) : 4 * (w % 2) + 4, :],
                    start=True,
                    stop=True,
                )

            # ---- pair-0 prologue ----
            xT_t = {0: emit_xT(0)}
            qk_t = {0: alloc_qk(0)}
            v_t = {0: alloc_v(0)}
            emit_qk(xT_t[0], *qk_t[0], 0)
            emit_qk(xT_t[0], *qk_t[0], 1)

            last_mm = None
            for q in range(PAIRS):
                xT = xT_t.pop(q)
                qk_sb, qk2_sb = qk_t.pop(q)
                vT_sb, vt = v_t.pop(q)

                # boundary: first scores, then this pair's first v half
                sc0 = emit_scores(qk_sb, qk2_sb, 0, 0)
                v_mm(xT, vT_sb, 0)

                # NOTE: vt[t]'s writes must be EMITTED before attnv(t)
                # (iteration t) or Tile resolves the read against the
                # previous pair's tile -- so v_tp(t0) sits at slot <= t0.
                f_h0 = {
                    0: [lambda: v_tp(vT_sb, vt, 0)],
                    1: [lambda: v_tp(vT_sb, vt, 2)],
                    2: [lambda: v_tp(vT_sb, vt, 4)],
                    3: [lambda: v_mm(xT, vT_sb, 1)],
                    4: [lambda: v_tp(vT_sb, vt, 6)],
                    5: [lambda: v_tp(vT_sb, vt, 8)],
                    6: [lambda: v_tp(vT_sb, vt, 10)],
                    7: [lambda: v_tp(vT_sb, vt, 12)],
                    8: [lambda: v_tp(vT_sb, vt, 14)],
                }
                if q + 1 < PAIRS:
                    f_h0[4].insert(
                        0, lambda: xT_t.__setitem__(q + 1, emit_xT(q + 1))
                    )
                    qk_t[q + 1] = alloc_qk(q + 1)
                    v_t[q + 1] = alloc_v(q + 1)
                    qkn, qk2n = qk_t[q + 1]
                    if q > 0:
                        f_h0[12] = [
                            lambda: emit_qk(xT_t[q + 1], qkn, qk2n, 0)
                        ]
                        f_h0[14] = [
                            lambda: emit_qk(xT_t[q + 1], qkn, qk2n, 1)
                        ]
                if q == 0:
                    # pair 0's own chunks 2/3 have in-half deadlines
                    # (kd tile t needs chunk t//4 by iteration t-1)
                    f_h0[2].append(lambda: emit_qk(xT, qk_sb, qk2_sb, 2))
                    f_h0[6].append(lambda: emit_qk(xT, qk_sb, qk2_sb, 3))
                run_half(q, 0, qk_sb, qk2_sb, vt, f_h0, sc0)

                sc0 = emit_scores(qk_sb, qk2_sb, 1, 0)
                f_h1 = {}
                if q + 1 < PAIRS:
                    qkn, qk2n = qk_t[q + 1]
                    for k in range(2):
                        f_h1[4 * k] = [
                            lambda n=2 + k: emit_qk(xT_t[q + 1], qkn, qk2n, n)
                        ]
                    if q == 0:
                        # pair-1 chunks 0/1 didn't fit in pair-0's h0
                        for k in range(2):
                            f_h1[8 + 4 * k] = [
                                lambda n=k: emit_qk(xT_t[q + 1], qkn, qk2n, n)
                            ]
                last_mm = run_half(q, 1, qk_sb, qk2_sb, vt, f_h1, sc0)

                if q == 1:
                    # head-slot 0 of every core is done: redistribute
                    nc.gpsimd.collective_compute(
                        "AllToAll",
                        mybir.AluOpType.bypass,
                        replica_groups=[list(range(N_CORES))],
                        ins=[a2a_in[0].opt()],
                        outs=[a2a_out[0].opt()],
                    )
                    # even-head rows of the gathered activations
                    for c in range(NDC):
                        nc.sync.dma_start(
                            out=asb[0:64, c, :], in_=a2a_out[0][c][:, :]
                        )

            # ---- head-slot-1 redistribution + output projection tail ----
            nc.gpsimd.collective_compute(
                "AllToAll",
                mybir.AluOpType.bypass,
                replica_groups=[list(range(N_CORES))],
                ins=[a2a_in[1].opt()],
                outs=[a2a_out[1].opt()],
            )
            # keep the PE clock gate open across the AllToAll wait
            for w in range(20):
                nc.tensor.matmul(
                    warm_ps[:],
                    wqk_sb[:, 0, 0:64],
                    wqk_sb[:, 4 * (w % 2) : 4 * (w % 2) + 4, :],
                    start=True,
                    stop=True,
                )
            for c in range(NDC):
                nc.sync.dma_start(
                    out=asb[64:128, c, :], in_=a2a_out[1][c][:, :]
                )

            oscope = nc.named_scope("outproj")
            oscope.__enter__()
            o_tiles = []
            for m in range(NDC):
                pool, tag = (ps_sc, "sc") if m % 2 == 0 else (
                    ps_proj, "proj_ps"
                )
                o_ps = pool.tile([128, SC], F32, tag=tag, name=f"o_ps{m}")
                for c in range(NDC):
                    mm = nc.tensor.matmul(
                        o_ps[:],
                        wo_sb[:, c, m, :],
                        asb[:, c, :],
                        start=(c == 0),
                        stop=(c == NDC - 1),
                    )
                    if last_mm is not None:
                        tile.add_dep_helper(
                            mm.ins, last_mm.ins, sync=False,
                            reason="outproj after attention",
                        )
                        last_mm = None
                o_sb = opool.tile([128, SC], F32, tag="o_sb")
                nc.vector.tensor_copy(o_sb[:], o_ps[:])
                nc.sync.dma_start(out=out[m][:, :], in_=o_sb[:])
                o_tiles.append(o_ps)
            oscope.__exit__(None, None, None)

    return nc


def _get_graph():
    global _GRAPH
    if _GRAPH is None:
        _GRAPH = _build_graph()
        if not _GRAPH.is_finalized():
            _GRAPH.finalize()
    return _GRAPH


def assemble(outs):
    # outs[r]: [8, 128, 512] f32 = out.T[:, 512r : 512r+512]
    full_t = np.concatenate(
        [np.asarray(o).reshape(D, SC) for o in outs], axis=1
    )  # [D, B*S]
    return np.ascontiguousarray(full_t.T).reshape(B, S, D)


def kernel(x, wq, wk, wv, wo):
    global LAST_RESULTS
    x = np.asarray(x, dtype=np.float32)
    wq = np.asarray(wq, dtype=np.float32)
    wk = np.asarray(wk, dtype=np.float32)
    wv = np.asarray(wv, dtype=np.float32)
    wo = np.asarray(wo, dtype=np.float32)

    bf16 = ml_dtypes.bfloat16
    # x transposed to [B, H, D, S] once (feeds matmuls as the moving operand)
    xt_all = np.ascontiguousarray(x.transpose(0, 1, 3, 2)).astype(bf16)
    wqk_t = np.ascontiguousarray(
        np.concatenate([wq, wk], axis=0).T
    ).astype(bf16)  # [D, 128]
    wv_t = np.ascontiguousarray(wv.T).astype(bf16)  # [D, 64]
    wv2_t = np.ascontiguousarray(
        np.concatenate([wv_t, wv_t], axis=1)
    )  # [D, 128] duplicated for col-tiled v projection
    wo_t = np.ascontiguousarray(wo.T).astype(bf16)  # [D, D], full per core

    in_maps = []
    for r in range(N_CORES):
        h0 = HPC * r
        # pair order: q = hl*B + b -> (b, h0+hl)
        xt_np = np.ascontiguousarray(
            xt_all[:, h0 : h0 + HPC]
            .transpose(1, 0, 2, 3)
            .reshape(PAIRS, D, S)
        )
        in_maps.append(
            {"xt": xt_np, "wqk": wqk_t, "wv2": wv2_t, "wo": wo_t}
        )

    nc = _get_graph()
    trace = bool(os.environ.get("BASS_TRACE"))
    if trace:
        try:  # tracing needs the axon NTFF hook; fall back cleanly
            from antenv.axon_hooks import get_axon_ntff_profile_hook  # noqa: F401
        except ImportError:
            trace = False
    tk = {}
    tc_env = os.environ.get("TRACE_CORES")
    if tc_env:
        tk["trace_cores"] = [int(c) for c in tc_env.split(",")]
    LAST_RESULTS = run_bass_kernel_spmd(
        nc, in_maps, core_ids=list(range(N_CORES)), trace=trace, **tk
    )
    outs = [LAST_RESULTS.results[r]["out"] for r in range(N_CORES)]
    return assemble(outs)
